# revision 1
# baseline (speedup 1.0000x reference)
"""Trainium2 Bass kernel for nn_MultiHeadAttention_89524298317897.

Data-parallel over batch: core b computes batch element b end-to-end
(no collectives). Inputs are pre-transposed/pre-scaled on the host so
every on-device matmul has its contraction dim on SBUF partitions.

Math per core (batch b), faithful to the reference's pure-reshape head
split (torch .view semantics chunk the sequence dim):
  qp = q @ (w_q/8).T ; kp = k @ w_k.T ; vp = v @ w_v.T
  per head h: A/B/C = rows [h*64,(h+1)*64) of qp/kp/vp reshaped [1024,64]
  U = A @ B.T ; E = exp(U) ; out_rows = (E @ C) / rowsum(E) -> reshape [64,1024]
  res = out @ w_o.T

Key layout trick: with T = A.T stored feature-major (qpT [o,s]), the
per-head operand T[:, (tsub,d) x (s_local)] decomposes into contiguous
64x64 blocks of qpT, so attention runs directly on the projection
outputs in a permuted token order (permutation-invariant through
softmax+PV; the output eviction un-permutes via addressing).

Matmuls run as float32r (1 cyc/row at N>=512). qp/kp/vp and exp(U) are
held in bf16; PSUM accumulation is fp32.
"""
import os
import sys

for _p in ("/opt/trn_rl_repo",):
    if os.path.isdir(_p) and _p not in sys.path:
        sys.path.insert(0, _p)

import numpy as np
import concourse.bass as bass
import concourse.mybir as mybir
import concourse.tile as tile
from concourse import bacc
from concourse.bass_utils import run_bass_kernel_spmd

B, S, D, NH, DH = 8, 1024, 1024, 16, 64
P = 128
F32 = mybir.dt.float32
F32R = mybir.dt.float32r
BF16 = mybir.dt.bfloat16
EXP_FN = mybir.ActivationFunctionType.Exp

_CACHE: dict = {}


def _build_nc(stage="full"):
    nc = bacc.Bacc("TRN2", target_bir_lowering=False, debug=False)

    qT = nc.dram_tensor("qT", [D, S], F32, kind="ExternalInput")
    kT = nc.dram_tensor("kT", [D, S], F32, kind="ExternalInput")
    vT = nc.dram_tensor("vT", [D, S], F32, kind="ExternalInput")
    wqT = nc.dram_tensor("wqT", [D, D], F32, kind="ExternalInput")
    wkT = nc.dram_tensor("wkT", [D, D], F32, kind="ExternalInput")
    wvT = nc.dram_tensor("wvT", [D, D], F32, kind="ExternalInput")
    woT = nc.dram_tensor("woT", [D, D], F32, kind="ExternalInput")
    out = nc.dram_tensor("out", [S, D], F32, kind="ExternalOutput")

    def part3(dram):  # [1024, X] -> [128, 8, X] with row = io*128 + p
        return dram[:].rearrange("(io p) x -> p io x", p=P)

    with tile.TileContext(nc) as tc:
        # ---- persistent outputs of phase A (bf16, 16KB/partition each) ----
        big_cm = tc.tile_pool(name="big", bufs=1)
        big = big_cm.__enter__()
        # qS: per-head-contiguous query layout. free = h*1024 + pi*512 +
        # oc*64 + s_l (query tsub = 2*oc + pi); partitions 0:64 AND 64:128
        # both hold d so scores rhs works at either base partition.
        qS = big.tile([P, NH, S], BF16)
        kA = big.tile([P, 8, S], BF16)     # kpT natural: [p, oc, s]
        vp_e = big.tile([P, 8, D], BF16)   # vp natural: [p, sc, o], s=sc*128+p
        vp_o = big.tile([P, 8, D], BF16)
        attn = big.tile([P, 8, S], F32)    # attn_outT natural: [p, jc, s]

        # ================= phase A: projections =================
        with tc.tile_pool(name="pa_x", bufs=1) as pa_x, \
             tc.tile_pool(name="pa_w", bufs=1) as pa_w, \
             tc.tile_pool(name="pa_ps", bufs=4, space="PSUM") as pa_ps:

            def project(x_dram, w_dram, evict, transpose_out):
                """PSUM <- (w.T @ x) tiles; evict(ps, mt, nchunk) stores."""
                xt = pa_x.tile([P, 8, S], F32, tag="x")
                wt = pa_w.tile([P, 8, D], F32, tag="w")
                nc.sync.dma_start(xt[:].bitcast(F32R),
                                  part3(x_dram).bitcast(F32R))
                nc.sync.dma_start(wt[:].bitcast(F32R),
                                  part3(w_dram).bitcast(F32R))
                for mt in range(8):          # output M tile (128 rows)
                    for nchunk in range(2):  # N chunk of 512
                        ps = pa_ps.tile([P, 512], F32, tag="ps")
                        for io in range(8):  # contraction over i
                            if transpose_out:
                                # qpT[o, s]: lhsT = w[:, io, o-tile], rhs = x
                                lhsT = wt[:, io, mt * P:(mt + 1) * P]
                                rhs = xt[:, io, nchunk * 512:(nchunk + 1) * 512]
                            else:
                                # vp[s, o]: lhsT = x[:, io, s-tile], rhs = w
                                lhsT = xt[:, io, mt * P:(mt + 1) * P]
                                rhs = wt[:, io, nchunk * 512:(nchunk + 1) * 512]
                            nc.tensor.matmul(
                                ps[:], lhsT.bitcast(F32R), rhs.bitcast(F32R),
                                start=(io == 0), stop=(io == 7))
                        evict(ps, mt, nchunk)

            def evict_natural(dst):
                def _e(ps, mt, nchunk):
                    nc.vector.tensor_copy(
                        dst[:, mt, nchunk * 512:(nchunk + 1) * 512], ps[:])
                return _e

            def evict_qS(ps, mt, nchunk):
                # psum M-tile mt = o-rows {tsub=2mt (lower), 2mt+1 (upper)};
                # s-chunk nchunk covers heads nchunk*8..+8, s_l 0..64.
                hs0 = nchunk * 8
                # lower: pi=0, oc=mt -> qS[0:64, h, mt*64 + s_l]
                nc.vector.tensor_copy(
                    qS[0:64, hs0:hs0 + 8, mt * 64:(mt + 1) * 64],
                    ps[0:64, :].rearrange("p (a b) -> p a b", a=8))
                # upper: pi=1, oc=mt -> qS[64:128, h, 512 + mt*64 + s_l]
                nc.vector.tensor_copy(
                    qS[64:128, hs0:hs0 + 8, 512 + mt * 64:512 + (mt + 1) * 64],
                    ps[64:128, :].rearrange("p (a b) -> p a b", a=8))

            project(qT, wqT, evict_qS, True)
            project(kT, wkT, evict_natural(kA), True)
            project(vT, wvT, evict_natural(vp_e), False)

            # partition-half swapped duplicates (SBUF->SBUF DMA)
            nc.sync.dma_start(qS[0:64, :, 512:1024], qS[64:128, :, 512:1024])
            nc.sync.dma_start(qS[64:128, :, 0:512], qS[0:64, :, 0:512])
            nc.sync.dma_start(vp_o[0:64], vp_e[64:128])
            nc.sync.dma_start(vp_o[64:128], vp_e[0:64])

        if stage == "proj":
            # debug: dump qS slices into out and stop
            with tc.tile_pool(name="dbg", bufs=2) as dbg:
                o3 = out[:].rearrange("(sc p) o -> p sc o", p=P)
                for scc in range(8):
                    t = dbg.tile([P, S], F32, tag="t")
                    nc.vector.tensor_copy(t[:], qS[:, scc * 2, :])
                    nc.sync.dma_start(o3[:, scc, :], t[:])

        # ================= phase B: attention =================
        # ET layout [128, 16 tsub_k, 512 t]: rows 0:64 = pi=0 queries,
        # rows 64:128 = pi=1. ET_swap = partition-halves-swapped DMA copy.
        # Quadrants: PV pi=0 @(row0,col0), PV pi=1 @(row0,col64),
        #            Z  pi=0 @(row64,col0), Z  pi=1 @(row64,col64).
        # Each PSUM tag keeps a FIXED partition range across iterations so
        # Tile's slot-reuse WAR tracking serializes bank reuse correctly.
        if stage not in ("proj",):
          with tc.tile_pool(name="pb_et", bufs=2) as pb_et, \
             tc.tile_pool(name="pb_sb", bufs=2) as pb_sb, \
             tc.tile_pool(name="pb_const", bufs=1) as pb_const, \
             tc.tile_pool(name="pb_sc", bufs=2, space="PSUM") as pb_sc, \
             tc.tile_pool(name="pb_num", bufs=1, space="PSUM") as pb_num:

            ones_bf = pb_const.tile([P, 64], BF16)
            nc.gpsimd.memset(ones_bf[:], 1.0)

            for h in range(NH):
                hs = slice(h * 64, (h + 1) * 64)
                ET = pb_et.tile([P, NH, 512], BF16, tag="ET")
                ETs = pb_et.tile([P, NH, 512], BF16, tag="ETs")
                # ---- scores + exp ----
                for c in range(8):           # tsub_k pair {2c, 2c+1}
                    ps = pb_sc.tile([P, 1024], F32, tag="sc")
                    for u in range(2):       # tsub_k = 2c+u, key parity u
                        tsub_k = 2 * c + u
                        lhsT = kA[u * 64:(u + 1) * 64, tsub_k // 2, hs]
                        for pi in range(2):
                            rhs = qS[u * 64:(u + 1) * 64, h,
                                     pi * 512:(pi + 1) * 512]
                            nc.tensor.matmul(
                                ps[pi * 64:(pi + 1) * 64,
                                   u * 512:(u + 1) * 512],
                                lhsT, rhs, start=True, stop=True)
                    nc.scalar.activation(
                        ET[:, 2 * c:2 * c + 2, :],
                        ps[:].rearrange("p (a b) -> p a b", a=2), EXP_FN)
                # swapped copy so Z groups can run on row tile 64
                nc.sync.dma_start(ETs[0:64], ET[64:128])
                nc.sync.dma_start(ETs[64:128], ET[0:64])

                for pi in range(2):
                    ns = slice(pi * 64, (pi + 1) * 64)
                    os_ = slice((1 - pi) * 64, (2 - pi) * 64)
                    nps = pb_num.tile([P, 512], F32, tag=f"num{pi}")
                    zps = pb_num.tile([P, 512], F32, tag=f"z{pi}")
                    # PV: row tile 0. pi=0 reads ET, pi=1 reads ETs.
                    E_pv = ET if pi == 0 else ETs
                    vp_x = vp_e if (h % 2) == 0 else vp_o
                    for i in range(NH):
                        lhsT = vp_x[0:64, h // 2, i * 64:(i + 1) * 64]
                        nc.tensor.matmul(nps[ns, :], lhsT, E_pv[0:64, i, :],
                                         start=(i == 0), stop=(i == NH - 1))
                    # Z: row tile 64. pi=0 reads ETs, pi=1 reads ET.
                    E_z = ETs if pi == 0 else ET
                    for i in range(NH):
                        nc.tensor.matmul(zps[ns, :], ones_bf[64:128, :],
                                         E_z[64:128, i, :],
                                         start=(i == 0), stop=(i == NH - 1))
                    # ---- evict: attn = num / z ----
                    probe = os.environ.get("PV_PROBE", "")
                    if probe == "num":
                        nc.vector.tensor_copy(
                            attn[ns, 0:8, hs],
                            nps[ns, :].rearrange("p (a b) -> p a b", a=8))
                    elif probe == "zps":
                        nc.vector.tensor_copy(
                            attn[ns, 0:8, hs],
                            zps[ns, :].rearrange("p (a b) -> p a b", a=8))
                    else:
                        # 1/Z = exp(-ln Z): two ACT table ops (exact DVE
                        # reciprocal costs ~6 cyc/lane-el; approx-fast DVE op
                        # produces garbage on HW).
                        zr_sb = pb_sb.tile([P, 512], F32, tag="zr")
                        zl_sb = pb_sb.tile([P, 512], F32, tag="zl")
                        nc.scalar.activation(zl_sb[ns, :], zps[ns, :],
                                             mybir.ActivationFunctionType.Ln)
                        nc.scalar.activation(zr_sb[ns, :], zl_sb[ns, :],
                                             EXP_FN, scale=-1.0)
                        nc.vector.tensor_tensor(
                            attn[ns, 0:8, hs],
                            nps[ns, :].rearrange("p (a b) -> p a b", a=8),
                            zr_sb[ns, :].rearrange("p (a b) -> p a b", a=8),
                            mybir.AluOpType.mult)

        if stage == "scores":
            with tc.tile_pool(name="dbg0", bufs=2) as dbg0:
                for kc in range(8):
                    t0 = dbg0.tile([P, S], F32, tag="t0")
                    nc.vector.tensor_copy(
                        t0[:].rearrange("p (a b) -> p a b", a=2),
                        ET[:, 2 * kc:2 * kc + 2, :])
                    nc.vector.tensor_copy(attn[:, kc, :], t0[:])

        if stage in ("attn", "scores"):
            with tc.tile_pool(name="dbg", bufs=2) as dbg:
                o3 = out[:].rearrange("(sc p) o -> p sc o", p=P)
                for scc in range(8):
                    t = dbg.tile([P, S], F32, tag="t")
                    nc.vector.tensor_copy(t[:], attn[:, scc, :])
                    nc.sync.dma_start(o3[:, scc, :], t[:])

        # ================= phase C: output projection =================
        if stage == "full":
          with tc.tile_pool(name="pc_w", bufs=1) as pc_w, \
             tc.tile_pool(name="pc_sb", bufs=3) as pc_sb, \
             tc.tile_pool(name="pc_ps", bufs=4, space="PSUM") as pc_ps:
            wo = pc_w.tile([P, 8, D], F32)
            nc.sync.dma_start(wo[:].bitcast(F32R), part3(woT).bitcast(F32R))
            attn_r = pc_w.tile([P, 8, S], F32R)
            nc.sync.dma_start(attn_r[:], attn[:].bitcast(F32R))
            out3 = out[:].rearrange("(sc p) o -> p sc o", p=P)
            for st in range(8):
                for oc in range(2):
                    ps = pc_ps.tile([P, 512], F32, tag="ps")
                    for jc in range(8):
                        nc.tensor.matmul(
                            ps[:],
                            attn_r[:, jc, st * P:(st + 1) * P],
                            wo[:, jc, oc * 512:(oc + 1) * 512].bitcast(F32R),
                            start=(jc == 0), stop=(jc == 7))
                    res = pc_sb.tile([P, 512], F32, tag="res")
                    nc.vector.tensor_copy(res[:], ps[:])
                    nc.sync.dma_start(out3[:, st, oc * 512:(oc + 1) * 512],
                                      res[:])

        big_cm.__exit__(None, None, None)

    nc.compile()
    return nc


def _get_nc():
    if "nc" not in _CACHE:
        _CACHE["nc"] = _build_nc()
    return _CACHE["nc"]


def kernel(q, k, v, mask, w_q, w_k, w_v, w_o, **_ignored):
    q = np.asarray(q, np.float32)
    k = np.asarray(k, np.float32)
    v = np.asarray(v, np.float32)
    wqT = np.ascontiguousarray((np.asarray(w_q, np.float32) / 8.0).T)
    wkT = np.ascontiguousarray(np.asarray(w_k, np.float32).T)
    wvT = np.ascontiguousarray(np.asarray(w_v, np.float32).T)
    woT = np.ascontiguousarray(np.asarray(w_o, np.float32).T)

    nc = _get_nc()
    in_maps = []
    for b in range(B):
        in_maps.append({
            "qT": np.ascontiguousarray(q[b].T),
            "kT": np.ascontiguousarray(k[b].T),
            "vT": np.ascontiguousarray(v[b].T),
            "wqT": wqT, "wkT": wkT, "wvT": wvT, "woT": woT,
        })
    res = run_bass_kernel_spmd(nc, in_maps, core_ids=list(range(B)))
    return np.stack([res.results[b]["out"] for b in range(B)]).astype(np.float32)



# revision 3
# speedup vs baseline: 2.1333x; 2.1333x over previous
"""Trainium2 Bass kernel for nn_MultiHeadAttention_89524298317897 (v2).

Data-parallel over batch: core b computes batch element b end-to-end.
All on-device tensors bf16 (host pre-casts + pre-transposes); PSUM
accumulation fp32.

Math per core (batch b), faithful to torch's .view head split (chunks
the sequence dim): head h token t <-> qp[64h + t//16, (t%16)*64 + d].
Key/query token order inside the kernel is the fixed permutation
m = t%16 = 2c + j, row = t//16  ->  free index f = j*512 + c*64 + row;
attention is permutation invariant and the output eviction un-permutes.

Layouts (per partition p):
  qS/kS [128, 16, 1024]  xS[64*jj + d, h, f] = xpT[(2c+j)*64+d, 64h+row]
        (qS only needs the diagonal halves; kS is dup'd via 2 swap DMAs)
  vS    [128, 16, 8, 64] vS[j*64 + krow, h, c, d] = vp[64h+krow, (2c+j)*64+d]
  ET    [128, 8, 2, 512] exp(scores) per head: [key-in-chunk, c, pi, q]
  attn  [128, 8, 1024]   out^T: attn[o%128, o//128, s]

Per head h: scores = 2 row-streams (pi) x 8 chunk MMs (K=64, M=128,
N=512) -> exp (ACT, N=1024 ops) -> PV: 2 col-streams x 8 accumulating
MMs (K=128) -> Z via ones lhsT -> DVE reciprocal + multiply eviction.
The head loop software-pipelines: scores(h)/exp(h) emitted alongside
PV(h-1), with v-projection and output-projection tiles interleaved so
the PE never idles while ACT computes exp (keeps the HAM clock warm).
"""
import os
import sys

for _p in ("/opt/trn_rl_repo",):
    if os.path.isdir(_p) and _p not in sys.path:
        sys.path.insert(0, _p)

import numpy as np
import ml_dtypes
import concourse.bass as bass
import concourse.mybir as mybir
import concourse.tile as tile
from concourse import bacc
from concourse.bass_utils import run_bass_kernel_spmd

B, S, D, NH, DH = 8, 1024, 1024, 16, 64
P = 128
F32 = mybir.dt.float32
BF16 = mybir.dt.bfloat16
EXP_FN = mybir.ActivationFunctionType.Exp

_CACHE: dict = {}


def _build_nc():
    nc = bacc.Bacc("TRN2", target_bir_lowering=False, debug=False)

    qT = nc.dram_tensor("qT", [D, S], BF16, kind="ExternalInput")
    kT = nc.dram_tensor("kT", [D, S], BF16, kind="ExternalInput")
    vT = nc.dram_tensor("vT", [D, S], BF16, kind="ExternalInput")
    wqT = nc.dram_tensor("wqT", [D, D], BF16, kind="ExternalInput")
    wkT = nc.dram_tensor("wkT", [D, D], BF16, kind="ExternalInput")
    wvT = nc.dram_tensor("wvT", [D, D], BF16, kind="ExternalInput")
    woT = nc.dram_tensor("woT", [D, D], BF16, kind="ExternalInput")
    out = nc.dram_tensor("out", [S, D], F32, kind="ExternalOutput")

    def part3(dram):  # [1024, X] -> [128, 8, X] with row = io*128 + p
        return dram[:].rearrange("(io p) x -> p io x", p=P)

    with tile.TileContext(nc) as tc:
        with tc.tile_pool(name="big", bufs=1) as big, \
             tc.tile_pool(name="pa_x", bufs=2) as pa_x, \
             tc.tile_pool(name="pa_w", bufs=2) as pa_w, \
             tc.tile_pool(name="pet", bufs=2) as pet, \
             tc.tile_pool(name="psb", bufs=2) as psb, \
             tc.tile_pool(name="pconst", bufs=1) as pconst, \
             tc.tile_pool(name="psc", bufs=2, space="PSUM") as psc, \
             tc.tile_pool(name="pnz", bufs=2, space="PSUM") as pnz:

            qS = big.tile([P, NH, S], BF16)
            kS = big.tile([P, NH, S], BF16)
            vS = big.tile([P, NH, 8, DH], BF16)
            attn = big.tile([P, 8, S], BF16)
            out3 = out[:].rearrange("(sc p) o -> p sc o", p=P)

            ones_bf = pconst.tile([P, 64], BF16)
            nc.gpsimd.memset(ones_bf[:], 1.0)

            # ---------- input DMAs (sync queue) ----------
            xq = pa_x.tile([P, 8, S], BF16, tag="x")
            wq = pa_w.tile([P, 8, D], BF16, tag="w")
            for io in range(8):  # chunked so first matmuls start early
                nc.sync.dma_start(wq[:, io, :], part3(wqT)[:, io, :])
                nc.sync.dma_start(xq[:, io, :], part3(qT)[:, io, :])
            xk = pa_x.tile([P, 8, S], BF16, tag="x")
            wk = pa_w.tile([P, 8, D], BF16, tag="w")
            nc.sync.dma_start(wk[:], part3(wkT))
            nc.sync.dma_start(xk[:], part3(kT))

            # ---------- phase A: q/k projections (transposed out) ----------
            def evict_qk(dst, ps_half, mt, nchunk):
                hs0 = nchunk * 8
                nc.vector.tensor_copy(
                    dst[0:64, hs0:hs0 + 8, mt * 64:(mt + 1) * 64],
                    ps_half[0:64].rearrange("p (a b) -> p a b", a=8))
                nc.vector.tensor_copy(
                    dst[64:128, hs0:hs0 + 8, 512 + mt * 64:512 + (mt + 1) * 64],
                    ps_half[64:128].rearrange("p (a b) -> p a b", a=8))

            for nchunk in range(2):
                for xt, wt, dst in ((xq, wq, qS), (xk, wk, kS)):
                    for j in range(4):
                        ps = psc.tile([P, 1024], F32, tag="sc")
                        for f in range(2):
                            mt = 2 * j + f
                            for io in range(8):
                                nc.tensor.matmul(
                                    ps[:, f * 512:(f + 1) * 512],
                                    wt[:, io, mt * P:(mt + 1) * P],
                                    xt[:, io, nchunk * 512:(nchunk + 1) * 512],
                                    start=(io == 0), stop=(io == 7))
                        for f in range(2):
                            evict_qk(dst, ps[:, f * 512:(f + 1) * 512],
                                     2 * j + f, nchunk)
                # kS needs both partition-half duplicates
                hs = slice(nchunk * 8, nchunk * 8 + 8)
                nc.gpsimd.dma_start(kS[64:128, hs, 0:512], kS[0:64, hs, 0:512])
                nc.gpsimd.dma_start(kS[0:64, hs, 512:1024],
                                    kS[64:128, hs, 512:1024])

            # ---------- v / wo loads ----------
            xv = pa_x.tile([P, 8, S], BF16, tag="x")
            wv = pa_w.tile([P, 8, D], BF16, tag="w")
            nc.sync.dma_start(wv[:], part3(wvT))
            nc.sync.dma_start(xv[:], part3(vT))
            wo = pa_w.tile([P, 8, D], BF16, tag="w")
            nc.sync.dma_start(wo[:], part3(woT))

            # ---------- helpers emitted inside the head loop ----------
            def emit_vproj(mtv):
                ps = psc.tile([P, 1024], F32, tag="sc")
                for f in range(2):  # nchunk
                    for io in range(8):
                        nc.tensor.matmul(
                            ps[:, f * 512:(f + 1) * 512],
                            xv[:, io, mtv * P:(mtv + 1) * P],
                            wv[:, io, f * 512:(f + 1) * 512],
                            start=(io == 0), stop=(io == 7))
                for f in range(2):
                    sc2 = psb.tile([P, 4, 64], BF16, tag="scratch")
                    psn = ps[:, f * 512:(f + 1) * 512]
                    lo = psn[0:64].rearrange("p (a e d) -> p a e d", a=4, e=2)
                    hi = psn[64:128].rearrange("p (a e d) -> p a e d", a=4, e=2)
                    cs = slice(4 * f, 4 * f + 4)
                    nc.vector.tensor_copy(vS[0:64, 2 * mtv, cs, :],
                                          lo[:, :, 0, :])
                    nc.vector.tensor_copy(sc2[0:64], lo[:, :, 1, :])
                    nc.vector.tensor_copy(sc2[64:128], hi[:, :, 0, :])
                    nc.vector.tensor_copy(vS[64:128, 2 * mtv + 1, cs, :],
                                          hi[:, :, 1, :])
                    nc.gpsimd.dma_start(vS[64:128, 2 * mtv, cs, :], sc2[0:64])
                    nc.gpsimd.dma_start(vS[0:64, 2 * mtv + 1, cs, :],
                                        sc2[64:128])

            def emit_phaseC(st):
                ps = psc.tile([P, 1024], F32, tag="sc")
                for oc in range(2):
                    for jc in range(8):
                        nc.tensor.matmul(
                            ps[:, oc * 512:(oc + 1) * 512],
                            attn[:, jc, st * P:(st + 1) * P],
                            wo[:, jc, oc * 512:(oc + 1) * 512],
                            start=(jc == 0), stop=(jc == 7))
                res = psb.tile([P, 1024], F32, tag="res")
                nc.vector.tensor_copy(res[:], ps[:])
                nc.gpsimd.dma_start(out3[:, st, :], res[:])

            # ---------- phase B: software-pipelined head loop ----------
            et_tiles = {}
            for h in range(NH + 1):
                if h < NH:
                    ET = pet.tile([P, 8, 2, 512], BF16, tag="et")
                    et_tiles[h] = ET
                    for c in range(8):
                        ps = psc.tile([P, 1024], F32, tag="sc")
                        for pi in range(2):
                            row = slice(64 * pi, 64 * pi + 64)
                            rhs = qS[row, h, pi * 512:(pi + 1) * 512]
                            for j in range(2):
                                lhsT = kS[row, h,
                                          j * 512 + c * 64:j * 512 + (c + 1) * 64]
                                nc.tensor.matmul(
                                    ps[64 * j:64 * j + 64,
                                       pi * 512:(pi + 1) * 512], lhsT, rhs,
                                    start=True, stop=True,
                                    skip_group_check=True)
                        nc.scalar.activation(
                            ET[:, c, :, :],
                            ps[:].rearrange("p (a b) -> p a b", a=2), EXP_FN)
                        if c == 3 and h % 2 == 0:
                            emit_vproj(h // 2)
                if h >= 1:
                    hp = h - 1
                    ETp = et_tiles.pop(hp)
                    num = pnz.tile([P, 512], F32, tag="num")
                    z = pnz.tile([P, 512], F32, tag="z")
                    for c in range(8):
                        for pi in range(2):
                            nc.tensor.matmul(
                                num[64 * pi:64 * pi + 64, :],
                                vS[:, hp, c, :], ET_rhs(ETp, c, pi),
                                start=(c == 0), stop=(c == 7),
                                skip_group_check=True)
                    for c in range(8):
                        for pi in range(2):
                            nc.tensor.matmul(
                                z[64 * pi:64 * pi + 64, :],
                                ones_bf[:], ET_rhs(ETp, c, pi),
                                start=(c == 0), stop=(c == 7),
                                skip_group_check=True)
                    zr = psb.tile([P, 512], F32, tag="zr")
                    nc.vector.reciprocal(zr[:], z[:])
                    nc.vector.tensor_tensor(
                        attn[:, 0:8, 64 * hp:64 * hp + 64],
                        num[:].rearrange("p (a b) -> p a b", a=8),
                        zr[:].rearrange("p (a b) -> p a b", a=8),
                        mybir.AluOpType.mult)
                    if hp % 2 == 1:
                        emit_phaseC(hp // 2)

    nc.compile()
    return nc


def ET_rhs(ET, c, pi):
    return ET[:, c, pi, :]


def _get_nc():
    if "nc" not in _CACHE:
        _CACHE["nc"] = _build_nc()
    return _CACHE["nc"]


def _prep_inputs(q, k, v, w_q, w_k, w_v, w_o):
    bf = ml_dtypes.bfloat16
    wqT = np.ascontiguousarray((np.asarray(w_q, np.float32) / 8.0).T).astype(bf)
    wkT = np.ascontiguousarray(np.asarray(w_k, np.float32).T).astype(bf)
    wvT = np.ascontiguousarray(np.asarray(w_v, np.float32).T).astype(bf)
    woT = np.ascontiguousarray(np.asarray(w_o, np.float32).T).astype(bf)
    in_maps = []
    for b in range(B):
        in_maps.append({
            "qT": np.ascontiguousarray(np.asarray(q[b], np.float32).T).astype(bf),
            "kT": np.ascontiguousarray(np.asarray(k[b], np.float32).T).astype(bf),
            "vT": np.ascontiguousarray(np.asarray(v[b], np.float32).T).astype(bf),
            "wqT": wqT, "wkT": wkT, "wvT": wvT, "woT": woT,
        })
    return in_maps


def kernel(q, k, v, mask, w_q, w_k, w_v, w_o, **_ignored):
    nc = _get_nc()
    in_maps = _prep_inputs(q, k, v, w_q, w_k, w_v, w_o)
    res = run_bass_kernel_spmd(nc, in_maps, core_ids=list(range(B)))
    return np.stack([res.results[b]["out"] for b in range(B)]).astype(np.float32)
